# revision 21
# baseline (speedup 1.0000x reference)
"""GCNConv custom kernel for Trainium2 (8 NeuronCores, SPMD row-sharded).

Math (matches the reference exactly):
    A = max(scatter(edges), scatter(edges).T) + I        # dense [N, N]
    deg = A.sum(axis=1); d = 1/sqrt(deg + EPS)
    out = (d[:,None] * A * d[None,:]) @ x @ W + b

Device d owns output rows [1024*d, 1024*(d+1)).  The host reformats
edge_index into the dense 0/1/2 adjacency slab for those rows (exact small
integers, fp8e4, laid out [j%128, half, j//128, li%512]) plus the integer
degrees' normalizers d = rsqrt(deg+eps); the O(N^2 d) math runs on device:

  z   = d * x                       (fp16, per-j-tile DVE scale)
  z8  = zhi (fp8e4) + zlo (fp8e5)   (hi/lo split keeps fp8 error ~1e-3)
  aggT[c, li] = sum_j z[j, c] * A_loc[li, j]
      -> DoubleRow PE matmuls (2 j-tiles per instruction, fp8 x fp8,
         0.5 cyc/row); per li-half one PSUM group accumulates a full zhi
         pass then a full zlo pass (so zlo production trails the casts)
  aggs = aggT * d_my[li]            (row scale; d_my tiled to [128,1024] by
                                     ones^T @ (ident (.) d_my) matmuls)
  out = aggs^T @ W + b              (per-li-tile matmul; bias rides the
                                     same PSUM group as a rank-1 matmul)

Performance structure: DMA transfer queues serialize per ISSUING engine,
so traffic is spread over three channels -- SP and ACT carry the adjacency
halves (ACT's issues interleaved one-per-cast-group so its sequencer can
do both), Pool carries the bulk of x; elementwise z8 production is spread
over DVE (scales + early zlo), ACT (zhi casts) and Pool (late zlo).  The
li columns are processed in two halves end-to-end so the first half's
row-scale / W-apply / copy-out / store hides under the second half's
matmuls.  All PSUM tiles share one pool.  No collectives.
"""

import sys

for _p in ("/root/.axon_site", "/root/.axon_site/_ro/trn_rl_repo", "/opt/trn_rl_repo"):
    if _p not in sys.path:
        sys.path.append(_p)

import ml_dtypes
import numpy as np

import concourse.bass as bass
import concourse.mybir as mybir
import concourse.tile as tile
from concourse import bacc
from concourse import bass_utils
from concourse.masks import make_identity

F32 = mybir.dt.float32
F16 = mybir.dt.float16
F8 = mybir.dt.float8e4
F8L = mybir.dt.float8e5

N = 8192
D = 128
NDEV = 8
NSH = N // NDEV          # rows per device
NT = N // 128            # j tiles
NL = NSH // 128          # li tiles
EPS = 1e-5
BCH = 4                  # j-tiles per adjacency DMA chunk (half-width)
NCH = NT // BCH          # chunks per li-half
ZG = 8                   # j-tiles per fp8-cast group
NG = NT // ZG            # cast groups


def _build_program(n=N, d=D, ndev=NDEV):
    """SPMD bass program; all per-core variation arrives as input data."""
    nsh = n // ndev
    nt = n // 128
    nl = nsh // 128
    hw = nsh // 2            # li-half width

    nc = bacc.Bacc("TRN2", target_bir_lowering=False, debug=False,
                   num_devices=ndev)

    xt_d = nc.dram_tensor("xt", [128, nt * d], F16, kind="ExternalInput")
    w_d = nc.dram_tensor("w", [d, d], F16, kind="ExternalInput")
    b_d = nc.dram_tensor("b", [1, d], F16, kind="ExternalInput")
    ablk_d = nc.dram_tensor("ablk", [128, nt * nsh], F8, kind="ExternalInput")
    dcol_d = nc.dram_tensor("dcol", [128, nt], F32, kind="ExternalInput")
    dmy_d = nc.dram_tensor("dmy", [128, nl], F32, kind="ExternalInput")
    out_d = nc.dram_tensor("out", [nsh, d], F16, kind="ExternalOutput")

    with tile.TileContext(nc) as tc:
        with (
            tc.tile_pool(name="const", bufs=1) as cpool,
            tc.tile_pool(name="blocks", bufs=1) as bpool,
            tc.tile_pool(name="psum", bufs=1, space="PSUM") as ppool,
        ):
            psum_md = ppool.tile([128, nsh], F32)
            psum_agg = ppool.tile([128, nsh], F32)
            psum_o = ppool.tile([128, nl, d], F32)

            # ---- x tile 0 and the normalizers first (they gate z) ----
            xz = cpool.tile([128, nt, d], F16)
            xv = xt_d.ap().rearrange("p (t c) -> p t c", c=d)
            nc.sync.dma_start(out=xz[:, 0:8, :], in_=xv[:, 0:8, :])
            dcol = cpool.tile([128, nt], F32)
            nc.sync.dma_start(out=dcol[:], in_=dcol_d.ap())
            dmy = cpool.tile([128, nl], F32)
            nc.sync.dma_start(out=dmy[:], in_=dmy_d.ap())
            # x tail: Pool carries t32-63 (one big SWDGE transfer, generated
            # before the identity build so it launches first), SP t8-31
            nc.gpsimd.dma_start(out=xz[:, 32:64, :], in_=xv[:, 32:64, :])
            nc.sync.dma_start(out=xz[:, 8:32, :], in_=xv[:, 8:32, :])
            wt = cpool.tile([128, d], F16)
            nc.sync.dma_start(out=wt[:], in_=w_d.ap())
            brow = cpool.tile([1, d], F16)
            nc.sync.dma_start(out=brow[:], in_=b_d.ap())
            ones1 = cpool.tile([128, d], F16)
            nc.vector.memset(ones1[:], 1.0)
            ident = cpool.tile([128, 128], F16)
            make_identity(nc, ident[:])

            # PE p-state warmup: rank-128 throwaway matmuls keep the PE busy
            # from ~1us so the aggregation starts at full clock
            psum_w = ppool.tile([128, 512], F32)
            for _ in range(8):
                nc.tensor.matmul(out=psum_w[:, 0:128], lhsT=ones1[:],
                                 rhs=ones1[:], start=True, stop=True)

            # adjacency slab [p, half, t, li']: half A = ACT (t0-31, woven
            # into the cast stream) + Pool (t32-63, big chunks); half B =
            # SP (t0-31, after x) + Pool (t32-63)
            blk = bpool.tile([128, 2, nt, hw], F8)
            av = ablk_d.ap().rearrange("p (s t l) -> p s t l", s=2, l=hw)

            def blk_dma(engine, half, t0, tn):
                engine.dma_start(out=blk[:, half, t0:t0 + tn, :],
                                 in_=av[:, half, t0:t0 + tn, :])

            blk_dma(nc.scalar, 0, 0, 4)               # ACT: first A chunks
            blk_dma(nc.scalar, 0, 4, 4)
            blk_dma(nc.gpsimd, 0, 32, 16)             # Pool: half A t 32..63
            blk_dma(nc.gpsimd, 0, 48, 16)
            for ci in range(8):                       # SP: half B t 0..31
                blk_dma(nc.sync, 1, ci * BCH, BCH)

            # ---- z pipeline ----
            # scales: one uninterrupted DVE burst (chasing the x chunks)
            zhi = cpool.tile([128, nt, d], F8)
            zlo = cpool.tile([128, nt, d], F8L)
            for t in range(nt):
                nc.vector.tensor_scalar_mul(
                    xz[:, t, :], xz[:, t, :], dcol[:, t:t + 1])
            # casts: ACT takes t0-35 (small first group for fast start,
            # remaining A-chunk issues woven between), DVE t36-47 after its
            # scale burst, Pool t48-63 after its DMA generation work
            act_groups = [(0, 4), (4, 12), (12, 20), (20, 28), (28, 36)]
            act_dmas = [(0, 8, 4), (0, 12, 4), (0, 16, 4),
                        (0, 20, 4), (0, 24, 4), (0, 28, 4)]
            for gi, (a0, a1) in enumerate(act_groups):
                nc.scalar.activation(
                    out=zhi[:, a0:a1, :], in_=xz[:, a0:a1, :],
                    func=mybir.ActivationFunctionType.Copy)
                for h_, t0_, tn_ in act_dmas[2 * gi:2 * gi + 2]:
                    blk_dma(nc.scalar, h_, t0_, tn_)
            nc.vector.tensor_copy(out=zhi[:, 36:48, :], in_=xz[:, 36:48, :])
            nc.gpsimd.tensor_copy(out=zhi[:, 48:64, :], in_=xz[:, 48:64, :])

            # mydbc[c, li] = d_my[li]: ones^T @ (ident (.) d_my_lt)
            diagm = cpool.tile([128, nl, 128], F16)
            mydbc = cpool.tile([128, nsh], F32)
            for lt in range(nl):
                nc.vector.tensor_scalar_mul(
                    diagm[:, lt, :], ident[:], dmy[:, lt:lt + 1])
            for lt in range(nl):
                nc.tensor.matmul(
                    out=psum_md[:, lt * 128:(lt + 1) * 128],
                    lhsT=ones1[:], rhs=diagm[:, lt, :],
                    start=True, stop=True)
            nc.scalar.activation(
                out=mydbc[:], in_=psum_md[:],
                func=mybir.ActivationFunctionType.Copy)
            # zlo subs: first half on DVE, second half on Pool
            for g0 in (0, 8, 16, 24):
                nc.vector.tensor_tensor(
                    out=zlo[:, g0:g0 + 8, :], in0=xz[:, g0:g0 + 8, :],
                    in1=zhi[:, g0:g0 + 8, :], op=mybir.AluOpType.subtract)
            for g0 in (32, 48):
                nc.gpsimd.tensor_tensor(
                    out=zlo[:, g0:g0 + 16, :], in0=xz[:, g0:g0 + 16, :],
                    in1=zhi[:, g0:g0 + 16, :], op=mybir.AluOpType.subtract)

            # Pool: half B t 32..63
            blk_dma(nc.gpsimd, 1, 32, 16)
            blk_dma(nc.gpsimd, 1, 48, 16)

            # ---- aggregation: per half, a full zhi pass then a full zlo
            # pass accumulate into one PSUM group (DoubleRow, 2 j-tiles/mm)
            ntp = nt // 2

            def agg_pass(half, z8, tps):
                for tp in tps:
                    t0 = 2 * tp
                    nc.tensor.matmul(
                        out=psum_agg[:, half * hw:(half + 1) * hw],
                        lhsT=z8[:, t0:t0 + 2, :],
                        rhs=blk[:, half, t0:t0 + 2, :],
                        perf_mode=mybir.MatmulPerfMode.DoubleRow,
                        start=(tp == 0 and z8 is zhi),
                        stop=(tp == ntp - 1 and z8 is zlo))

            aggs = cpool.tile([128, nsh], F16)
            o_all = cpool.tile([128, nl, d], F16)
            ov = out_d.ap().rearrange("(t p) c -> p t c", p=128)

            def tail(half):
                l0 = half * (nl // 2)
                l1 = l0 + nl // 2
                nc.vector.tensor_tensor(
                    out=aggs[:, l0 * 128:l1 * 128],
                    in0=psum_agg[:, l0 * 128:l1 * 128],
                    in1=mydbc[:, l0 * 128:l1 * 128],
                    op=mybir.AluOpType.mult)
                for lt in range(l0, l1):
                    nc.tensor.matmul(
                        out=psum_o[:, lt, :],
                        lhsT=aggs[:, lt * 128:(lt + 1) * 128],
                        rhs=wt[:], start=True, stop=False)
                    nc.tensor.matmul(
                        out=psum_o[:, lt, :],
                        lhsT=ones1[0:1, :], rhs=brow[:],
                        start=False, stop=True)
                nc.scalar.activation(
                    out=o_all[:, l0:l1, :], in_=psum_o[:, l0:l1, :],
                    func=mybir.ActivationFunctionType.Copy)
                nc.sync.dma_start(out=ov[:, l0:l1, :], in_=o_all[:, l0:l1, :])

            agg_pass(0, zhi, range(ntp))
            agg_pass(0, zlo, range(ntp))
            agg_pass(1, zhi, range(8))
            tail(0)
            agg_pass(1, zhi, range(8, ntp))
            agg_pass(1, zlo, range(ntp))
            tail(1)

    nc.compile()
    return nc


_F8LUT = np.array([0.0, 1.0, 2.0], dtype=ml_dtypes.float8_e4m3fn).view(np.uint8)


def _host_prep(x, edge_index, weight, bias, n=N, ndev=NDEV):
    """Reformat edge_index into per-device dense fp8 adjacency slabs plus
    the degree normalizers (graph structure; the O(N^2 d) math is on device)."""
    nsh = n // ndev
    nt = n // 128
    nl = nsh // 128
    d = x.shape[1]

    a = np.asarray(edge_index[0], dtype=np.int64)
    b = np.asarray(edge_index[1], dtype=np.int64)

    m = np.zeros((n, n), dtype=np.uint8)
    m[a, b] = 1
    np.maximum(m, m.T, out=m)            # symmetrize
    idx = np.arange(n)
    m[idx, idx] += 1                     # self-loops (may yield 2 on diag)
    deg = m.sum(axis=1, dtype=np.int32).astype(np.float32)
    dns = (1.0 / np.sqrt(deg + np.float32(EPS))).astype(np.float32)

    x = np.asarray(x, dtype=np.float32)
    # [p, t, c] fp16 layout (relayout + the same cast the device DMA did)
    xtp = np.ascontiguousarray(x.reshape(nt, 128, d).transpose(1, 0, 2)
                               .astype(np.float16)).reshape(128, nt * d)
    w = np.ascontiguousarray(np.asarray(weight, dtype=np.float16))
    bias = np.ascontiguousarray(
        np.asarray(bias, dtype=np.float16)).reshape(1, -1)
    dcol = np.ascontiguousarray(dns.reshape(nt, 128).T)

    in_maps = []
    for dv in range(ndev):
        md = m[dv * nsh:(dv + 1) * nsh]                    # [nsh, n] {0,1,2}
        # ablk[p, half, t, li'] = A[dv*nsh + half*512 + li', t*128 + p]
        ab = _F8LUT[md.reshape(2, nsh // 2, nt, 128).transpose(3, 0, 2, 1)]
        ab = np.ascontiguousarray(ab.reshape(128, nt * nsh)).view(
            ml_dtypes.float8_e4m3fn)
        in_maps.append({
            "xt": xtp, "w": w, "b": bias,
            "ablk": ab,
            "dcol": dcol,
            "dmy": np.ascontiguousarray(
                dns[dv * nsh:(dv + 1) * nsh].reshape(nl, 128).T),
        })
    return in_maps


_prog_cache = {}


def _get_program():
    key = (N, D, NDEV)
    if key not in _prog_cache:
        _prog_cache[key] = _build_program()
    return _prog_cache[key]


last_results = None
TRACE = False


def kernel(x, edge_index, weight, bias):
    global last_results
    in_maps = _host_prep(x, edge_index, weight, bias)
    nc = _get_program()
    res = bass_utils.run_bass_kernel_spmd(
        nc, in_maps, core_ids=list(range(NDEV)), trace=TRACE)
    last_results = res
    out = np.concatenate([res.results[i]["out"] for i in range(NDEV)], axis=0)
    return out.astype(np.float32)
